# revision 10
# baseline (speedup 1.0000x reference)
"""Trainium2 Bass kernel for the STFT patch-dispatch loss.

Math (matches the reference exactly, in fp32):
  For each of 3 waveforms x[B=16, L=262144]:
    xp = reflect_pad(x, 512)                      # [263168] = 1028 blocks of 256
    V[r, m] = xp[256 m + r]                       # on-chip via PE transpose
    Block DFT (one fp32 matmul pair per freq-class chunk):
      B_m[k] = sum_r V[r, m] e^{-2 pi i k r / 1024}
    Radix-4 recombination with twiddles (-i)^{k j} (pure adds, on DVE):
      X_t[k] = sum_{j=0..3} (-i)^{k j} B_{t+j}[k]
    mag = sqrt(re^2 + im^2)                       # [513, 1025]
  Patch sums over 16x16 patches of |s-g|, |t-g|, (s-t)^2 -> [33, 65] per map.
  Host: top-k mask + final scalar reductions (tiny).

Frequencies are permuted into residue classes mod 4 (chunk c holds k = 4p+c)
so the recombination weights are uniform per chunk; k=512 (Nyquist) is a
separate 1-row matmul.  Sharding: batch rows 2c, 2c+1 -> core c (8 cores).
"""
import numpy as np

import concourse.bass as bass
import concourse.bacc as bacc
import concourse.mybir as mybir
from concourse import tile

dt = mybir.dt
Alu = mybir.AluOpType
Act = mybir.ActivationFunctionType

B, L = 16, 262144
NCORES = 8
RPC = B // NCORES          # rows per core
NFFT, HOP, PS = 1024, 256, 16
PAD = NFFT // 2            # 512
LP = L + 2 * PAD           # 263168
NBLK = LP // HOP           # 1028
T = 1 + (LP - NFFT) // HOP  # 1025 frames
NF = 513                   # onesided freqs
NPF, NPT = 33, 65          # patch grid
KSEL = max(1, int(NPF * NPT * 0.3))  # 643

M_RANGES = [(0, 512), (512, 1024), (1024, NBLK)]


def _consts():
    r = np.arange(256)
    p = np.arange(128)
    wc = np.empty((256, 512), np.float32)
    ws = np.empty((256, 512), np.float32)
    for c in range(4):
        k = 4 * p + c  # freqs of chunk c
        ang = 2.0 * np.pi * np.outer(r, k) / NFFT
        wc[:, 128 * c:128 * (c + 1)] = np.cos(ang)
        ws[:, 128 * c:128 * (c + 1)] = -np.sin(ang)
    wn = np.where(r % 2 == 0, 1.0, -1.0).astype(np.float32).reshape(256, 1)
    ones4 = (p[:, None] // 4 == np.arange(32)[None, :]).astype(np.float32)
    ident = np.eye(128, dtype=np.float32)
    return {
        "wc0": wc[:128], "wc1": wc[128:],
        "ws0": ws[:128], "ws1": ws[128:],
        "wn0": wn[:128], "wn1": wn[128:],
        "ones4": ones4, "ident": ident,
    }


CONST_SPECS = {
    "wc0": [128, 512], "wc1": [128, 512],
    "ws0": [128, 512], "ws1": [128, 512],
    "wn0": [128, 1], "wn1": [128, 1],
    "ones4": [128, 32], "ident": [128, 128],
}


def _seg(x_d, b, start, nrows):
    """[nrows, 256] DRAM view of x row b at sample offset `start`."""
    return x_d[b:b + 1, start:start + 256 * nrows].rearrange(
        "o (m r) -> (o m) r", r=256)


def build_nc():
    nc = bacc.Bacc("TRN2", target_bir_lowering=False, debug=False,
                   num_devices=NCORES)

    x_d = {s: nc.dram_tensor(f"x{s}", [RPC, L], dt.float32,
                             kind="ExternalInput") for s in "stg"}
    c_d = {n: nc.dram_tensor(n, shp, dt.float32, kind="ExternalInput")
           for n, shp in CONST_SPECS.items()}
    osum_d = nc.dram_tensor("osum", [RPC * 3, NPF, NPT], dt.float32,
                            kind="ExternalOutput")

    with tile.TileContext(nc) as tc:
        with (
            tc.tile_pool(name="const", bufs=1) as cp,
            tc.tile_pool(name="work", bufs=2) as wp,
            tc.tile_pool(name="upool", bufs=4) as up,
            tc.tile_pool(name="mpool", bufs=13) as mp,
            tc.tile_pool(name="mnpool", bufs=2) as mnp,
            tc.tile_pool(name="dft_ps", bufs=3, space="PSUM") as dft_ps,
            tc.tile_pool(name="tr_ps", bufs=2, space="PSUM") as tr_ps,
            tc.tile_pool(name="ny_ps", bufs=1, space="PSUM") as ny_ps,
            tc.tile_pool(name="pa_ps", bufs=2, space="PSUM") as pa_ps,
        ):
            C = {n: cp.tile(shp, dt.float32, tag=n, name=f"c_{n}")
                 for n, shp in CONST_SPECS.items()}
            for n in CONST_SPECS:
                nc.sync.dma_start(C[n][:], c_d[n][:])

            def tr_to_v(u_ap, v0, v1, col, nr):
                """Transpose [nr, 256] rows into V columns [col, col+nr)."""
                for h, vt in ((0, v0), (1, v1)):
                    tp = tr_ps.tile([128, 128], dt.float32, tag="trp",
                                    name="tp")
                    nc.tensor.transpose(
                        tp[:, 0:nr], u_ap[:, 128 * h:128 * h + 128],
                        C["ident"][0:nr, 0:nr])
                    nc.scalar.copy(vt[:, col:col + nr], tp[:, 0:nr])

            def rev2(name, hi_start, lo_start, s, b):
                """[2,256] tile: row0=rev(x[hi:hi+256]), row1=rev(x[lo:..])."""
                sc = up.tile([2, 256], dt.float32, tag="sc", name="sc")
                nc.sync.dma_start(sc[0:1, :],
                                  x_d[s][b:b + 1, hi_start:hi_start + 256])
                nc.sync.dma_start(sc[1:2, :],
                                  x_d[s][b:b + 1, lo_start:lo_start + 256])
                ur = up.tile([2, 256], dt.float32, tag="ur", name="ur")
                nc.vector.tensor_copy(ur[:], sc[0:2, 255::-1])
                return ur

            def build_V(s, b):
                """Load row b of signal s, reflect-pad, transpose to
                V[r, m] (two [128, NBLK] tiles, r-halves)."""
                v0 = wp.tile([128, NBLK], dt.float32, tag="v0")
                v1 = wp.tile([128, NBLK], dt.float32, tag="v1")
                # head reflect: U[0,r]=x[512-r]=rev(x[257:513]);
                #               U[1,r]=x[256-r]=rev(x[1:257])
                uh = rev2("uh", 257, 1, s, b)
                tr_to_v(uh[0:2, :], v0, v1, 0, 2)
                u0 = up.tile([128, 256], dt.float32, tag="u", name="u0")
                nc.sync.dma_start(u0[0:126, :], _seg(x_d[s], b, 0, 126))
                tr_to_v(u0[0:126, :], v0, v1, 2, 126)
                for i in range(1, 8):
                    u = up.tile([128, 256], dt.float32, tag="u", name="u")
                    nc.sync.dma_start(
                        u[:], _seg(x_d[s], b, 32768 * i - 512, 128))
                    tr_to_v(u[:], v0, v1, 128 * i, 128)
                # tail: blocks 1024,1025 contiguous
                ut = up.tile([2, 256], dt.float32, tag="ur", name="ut")
                nc.sync.dma_start(ut[:], _seg(x_d[s], b, 261632, 2))
                tr_to_v(ut[0:2, :], v0, v1, 1024, 2)
                # tail reflect: U[1026,r]=x[262142-r]=rev(x[261887:262143]);
                #               U[1027,r]=x[261886-r]=rev(x[261631:261887])
                ub = rev2("ub", 261887, 261631, s, b)
                tr_to_v(ub[0:2, :], v0, v1, 1026, 2)
                return v0, v1

            def stft_mag(s, b):
                """mag chunks [128, T] x4 (freq k=4p+c) + nyquist [1, T]."""
                v0, v1 = build_V(s, b)
                mags = []
                for c in range(4):
                    rc = wp.tile([128, NBLK], dt.float32, tag="rc")
                    ic = wp.tile([128, NBLK], dt.float32, tag="ic")
                    for (w0, w1, dst) in (("wc0", "wc1", rc),
                                          ("ws0", "ws1", ic)):
                        for lo, hi in M_RANGES:
                            ps = dft_ps.tile([128, hi - lo], dt.float32,
                                             tag="dftp")
                            nc.tensor.matmul(
                                ps[:], C[w0][:, 128 * c:128 * (c + 1)],
                                v0[:, lo:hi], start=True, stop=False)
                            nc.tensor.matmul(
                                ps[:], C[w1][:, 128 * c:128 * (c + 1)],
                                v1[:, lo:hi], start=False, stop=True)
                            nc.scalar.copy(dst[:, lo:hi], ps[:])
                    # radix-4 recombination (pure adds)
                    pr = wp.tile([128, NBLK - 2], dt.float32, tag="pr")
                    pi = wp.tile([128, NBLK - 2], dt.float32, tag="pi")
                    op2 = nc.vector.tensor_add if c % 2 == 0 else \
                        nc.vector.tensor_sub
                    op2(pr[:], rc[:, 0:NBLK - 2], rc[:, 2:NBLK])
                    op2(pi[:], ic[:, 0:NBLK - 2], ic[:, 2:NBLK])
                    xre = wp.tile([128, T], dt.float32, tag="xre")
                    xim = wp.tile([128, T], dt.float32, tag="xim")
                    if c == 0:
                        nc.vector.tensor_add(xre[:], pr[:, 0:T], pr[:, 1:T + 1])
                        nc.vector.tensor_add(xim[:], pi[:, 0:T], pi[:, 1:T + 1])
                    elif c == 2:
                        nc.vector.tensor_sub(xre[:], pr[:, 0:T], pr[:, 1:T + 1])
                        nc.vector.tensor_sub(xim[:], pi[:, 0:T], pi[:, 1:T + 1])
                    elif c == 1:
                        nc.vector.tensor_add(xre[:], pr[:, 0:T], pi[:, 1:T + 1])
                        nc.vector.tensor_sub(xim[:], pi[:, 0:T], pr[:, 1:T + 1])
                    else:
                        nc.vector.tensor_sub(xre[:], pr[:, 0:T], pi[:, 1:T + 1])
                        nc.vector.tensor_add(xim[:], pi[:, 0:T], pr[:, 1:T + 1])
                    # in-place: xre <- xre^2, xim <- xim^2, xre <- xre+xim
                    nc.scalar.activation(xre[:], xre[:], Act.Square)
                    nc.scalar.activation(xim[:], xim[:], Act.Square)
                    nc.vector.tensor_add(xre[:], xre[:], xim[:])
                    mg = mp.tile([128, T], dt.float32, tag="mag")
                    nc.scalar.activation(mg[:], xre[:], Act.Sqrt)
                    mags.append(mg)
                # Nyquist row: k=512, im = 0, class-0 recombination
                bn = mnp.tile([1, NBLK], dt.float32, tag="bn", bufs=2)
                for lo, hi in M_RANGES:
                    ps = ny_ps.tile([1, hi - lo], dt.float32, tag="nyp")
                    nc.tensor.matmul(ps[:], C["wn0"][:, 0:1], v0[:, lo:hi],
                                     start=True, stop=False)
                    nc.tensor.matmul(ps[:], C["wn1"][:, 0:1], v1[:, lo:hi],
                                     start=False, stop=True)
                    nc.vector.tensor_copy(bn[0:1, lo:hi], ps[:])
                an = mnp.tile([1, NBLK - 2], dt.float32, tag="an", bufs=2)
                nc.vector.tensor_add(an[:], bn[0:1, 0:NBLK - 2],
                                     bn[0:1, 2:NBLK])
                mn = mnp.tile([1, T], dt.float32, tag="magn", bufs=4)
                nc.vector.tensor_add(mn[:], an[0:1, 0:T], an[0:1, 1:T + 1])
                nc.scalar.activation(mn[:], mn[:], Act.Abs)
                return mags, mn

            def patch_map(b, mi, ma, mb, na, nb, square):
                """osum[b*3+mi] = per-patch sums of |A-B| or (A-B)^2."""
                pps = pa_ps.tile([32, NPT], dt.float32, tag="pps")
                for c in range(4):
                    d = wp.tile([128, T], dt.float32, tag="d")
                    nc.vector.tensor_sub(d[:], ma[c][:], mb[c][:])
                    if square:
                        nc.scalar.activation(d[:], d[:], Act.Square)
                        src, ab = d, False
                    else:
                        src, ab = d, True
                    red = wp.tile([128, NPT], dt.float32, tag="red")
                    nc.vector.tensor_reduce(
                        red[:, 0:64],
                        src[:, 0:1024].rearrange("p (a t) -> p a t", t=16),
                        axis=mybir.AxisListType.X, op=Alu.add,
                        apply_absolute_value=ab)
                    nc.vector.tensor_reduce(
                        red[:, 64:65], src[:, 1024:1025],
                        axis=mybir.AxisListType.X, op=Alu.add,
                        apply_absolute_value=ab)
                    nc.tensor.matmul(pps[:], C["ones4"][:], red[:],
                                     start=(c == 0), stop=(c == 3))
                # nyquist row -> patch row 32
                dn = mnp.tile([1, T], dt.float32, tag="dn", bufs=2)
                nc.vector.tensor_sub(dn[:], na[:], nb[:])
                if square:
                    nc.scalar.activation(dn[:], dn[:], Act.Square)
                    srcn, ab = dn, False
                else:
                    srcn, ab = dn, True
                outt = wp.tile([NPF, NPT], dt.float32, tag="outt")
                nc.vector.tensor_reduce(
                    outt[32:33, 0:64],
                    srcn[0:1, 0:1024].rearrange("p (a t) -> p a t", t=16),
                    axis=mybir.AxisListType.X, op=Alu.add,
                    apply_absolute_value=ab)
                nc.vector.tensor_reduce(
                    outt[32:33, 64:65], srcn[0:1, 1024:1025],
                    axis=mybir.AxisListType.X, op=Alu.add,
                    apply_absolute_value=ab)
                nc.vector.tensor_copy(outt[0:32, :], pps[:])
                idx = b * 3 + mi
                nc.sync.dma_start(
                    osum_d[idx:idx + 1].rearrange("o p f -> (o p) f"),
                    outt[:])

            for b in range(RPC):
                ms, nys = stft_mag("s", b)
                mt, nyt = stft_mag("t", b)
                mg_, nyg = stft_mag("g", b)
                patch_map(b, 0, ms, mg_, nys, nyg, False)
                patch_map(b, 1, mt, mg_, nyt, nyg, False)
                patch_map(b, 2, ms, mt, nys, nyt, True)

    nc.compile()
    return nc


_NC_CACHE = {}


def _get_nc():
    if "nc" not in _NC_CACHE:
        _NC_CACHE["nc"] = build_nc()
    return _NC_CACHE["nc"]


def _run_on_cores(nc, in_maps):
    """Execute via cached PJRT callable (axon) with jit reuse."""
    from concourse.bass_utils import axon_active

    if not axon_active():
        from concourse.bass_utils import run_bass_kernel_spmd
        return run_bass_kernel_spmd(nc, in_maps,
                                    core_ids=list(range(NCORES))).results

    import jax
    from jax.sharding import Mesh, PartitionSpec
    from jax.experimental.shard_map import shard_map
    from concourse import bass2jax

    key = id(nc)
    if key not in _NC_CACHE.setdefault("jit", {}):
        bass2jax.install_neuronx_cc_hook()
        part_name = (nc.partition_id_tensor.name
                     if nc.partition_id_tensor else None)
        in_names, out_names, out_avals, zero_outs = [], [], [], []
        for alloc in nc.m.functions[0].allocations:
            if not isinstance(alloc, mybir.MemoryLocationSet):
                continue
            name = alloc.memorylocations[0].name
            if alloc.kind == "ExternalInput":
                if name != part_name:
                    in_names.append(name)
            elif alloc.kind == "ExternalOutput":
                shape = tuple(alloc.tensor_shape)
                dtype = mybir.dt.np(alloc.dtype)
                out_names.append(name)
                out_avals.append(jax.core.ShapedArray(shape, dtype))
                zero_outs.append(np.zeros(shape, dtype))
        n_params = len(in_names)
        all_names = in_names + out_names
        if part_name is not None:
            all_names = all_names + [part_name]

        def _body(*args):
            operands = list(args)
            if part_name is not None:
                operands.append(bass2jax.partition_id_tensor())
            outs = bass2jax._bass_exec_p.bind(
                *operands, out_avals=tuple(out_avals),
                in_names=tuple(all_names), out_names=tuple(out_names),
                lowering_input_output_aliases=(),
                sim_require_finite=True, sim_require_nnan=True, nc=nc)
            return tuple(outs)

        devices = jax.devices()[:NCORES]
        mesh = Mesh(np.asarray(devices), ("core",))
        n_outs = len(out_names)
        sharded = jax.jit(
            shard_map(_body, mesh=mesh,
                      in_specs=(PartitionSpec("core"),) * (n_params + n_outs),
                      out_specs=(PartitionSpec("core"),) * n_outs,
                      check_rep=False),
            donate_argnums=tuple(range(n_params, n_params + n_outs)),
            keep_unused=True)
        _NC_CACHE["jit"][key] = (sharded, in_names, out_names, out_avals,
                                 zero_outs)

    sharded, in_names, out_names, out_avals, zero_outs = _NC_CACHE["jit"][key]
    concat_in = [np.concatenate([m[n] for m in in_maps], axis=0)
                 for n in in_names]
    concat_zeros = [np.zeros((NCORES * z.shape[0], *z.shape[1:]), z.dtype)
                    for z in zero_outs]
    out_arrs = sharded(*concat_in, *concat_zeros)
    return [
        {n: np.asarray(out_arrs[i]).reshape(NCORES, *out_avals[i].shape)[c]
         for i, n in enumerate(out_names)}
        for c in range(NCORES)
    ]


def kernel(student_waveform, teacher_waveform, target_waveform,
           n_fft=1024, hop_length=256, patch_size=16):
    xs = np.ascontiguousarray(student_waveform, dtype=np.float32)
    xt = np.ascontiguousarray(teacher_waveform, dtype=np.float32)
    xg = np.ascontiguousarray(target_waveform, dtype=np.float32)

    nc = _get_nc()
    consts = _consts()
    in_maps = []
    for c in range(NCORES):
        m = {"xs": xs[RPC * c:RPC * (c + 1)],
             "xt": xt[RPC * c:RPC * (c + 1)],
             "xg": xg[RPC * c:RPC * (c + 1)]}
        m.update(consts)
        in_maps.append(m)

    results = _run_on_cores(nc, in_maps)

    # [B, 3, NPF, NPT] patch sums
    osum = np.concatenate(
        [r["osum"].reshape(RPC, 3, NPF, NPT) for r in results], axis=0)
    sums = osum.reshape(B, 3, NPF * NPT).astype(np.float32)
    inv = np.float32(1.0 / (PS * PS))
    err_s = sums[:, 0] * inv
    err_t = sums[:, 1] * inv
    pl = sums[:, 2] * inv
    kgs = err_s - err_t

    order = np.argsort(-kgs, axis=1, kind="stable")[:, :KSEL]
    mask = np.zeros_like(kgs)
    np.put_along_axis(mask, order, 1.0, axis=1)
    selected = (pl * mask).sum(axis=1, dtype=np.float32)
    count = np.maximum(mask.sum(axis=1, dtype=np.float32), 1.0)
    loss = np.float32(np.mean(selected / count, dtype=np.float32))
    sel_ratio = np.float32(mask.mean(dtype=np.float32))
    kgs_mean = np.float32(kgs.mean(dtype=np.float32))
    kgs_pos_ratio = np.float32((kgs > 0).mean(dtype=np.float32))
    return loss, sel_ratio, kgs_mean, kgs_pos_ratio


# revision 12
# speedup vs baseline: 314.9748x; 314.9748x over previous
"""Trainium2 Bass kernel for the STFT patch-dispatch loss.

Math (matches the reference exactly, in fp32):
  For each of 3 waveforms x[B=16, L=262144]:
    xp = reflect_pad(x, 512)                      # [263168] = 1028 blocks of 256
    V[r, m] = xp[256 m + r]                       # on-chip via PE transpose
    Block DFT (one fp32 matmul pair per freq-class chunk):
      B_m[k] = sum_r V[r, m] e^{-2 pi i k r / 1024}
    Radix-4 recombination with twiddles (-i)^{k j} (pure adds, on DVE):
      X_t[k] = sum_{j=0..3} (-i)^{k j} B_{t+j}[k]
    mag = sqrt(re^2 + im^2)                       # [513, 1025]
  Patch sums over 16x16 patches of |s-g|, |t-g|, (s-t)^2 -> [33, 65] per map.
  Host: top-k mask + final scalar reductions (tiny).

Frequencies are permuted into residue classes mod 4 (chunk c holds k = 4p+c)
so the recombination weights are uniform per chunk; k=512 (Nyquist) is a
separate 1-row matmul.  Sharding: batch rows 2c, 2c+1 -> core c (8 cores).
"""
import numpy as np

import concourse.bass as bass
import concourse.bacc as bacc
import concourse.mybir as mybir
from concourse import tile

dt = mybir.dt
Alu = mybir.AluOpType
Act = mybir.ActivationFunctionType

B, L = 16, 262144
NCORES = 8
RPC = B // NCORES          # rows per core
NFFT, HOP, PS = 1024, 256, 16
PAD = NFFT // 2            # 512
LP = L + 2 * PAD           # 263168
NBLK = LP // HOP           # 1028
T = 1 + (LP - NFFT) // HOP  # 1025 frames
NF = 513                   # onesided freqs
NPF, NPT = 33, 65          # patch grid
KSEL = max(1, int(NPF * NPT * 0.3))  # 643

M_RANGES = [(0, 512), (512, 1024), (1024, NBLK)]


def _consts():
    r = np.arange(256)
    p = np.arange(128)
    wc = np.empty((256, 512), np.float32)
    ws = np.empty((256, 512), np.float32)
    for c in range(4):
        k = 4 * p + c  # freqs of chunk c
        ang = 2.0 * np.pi * np.outer(r, k) / NFFT
        wc[:, 128 * c:128 * (c + 1)] = np.cos(ang)
        ws[:, 128 * c:128 * (c + 1)] = -np.sin(ang)
    wn = np.where(r % 2 == 0, 1.0, -1.0).astype(np.float32).reshape(256, 1)
    ones4 = (p[:, None] // 4 == np.arange(32)[None, :]).astype(np.float32)
    ident = np.eye(128, dtype=np.float32)
    return {
        "wc0": wc[:128], "wc1": wc[128:],
        "ws0": ws[:128], "ws1": ws[128:],
        "wn0": wn[:128], "wn1": wn[128:],
        "ones4": ones4, "ident": ident,
    }


CONST_SPECS = {
    "wc0": [128, 512], "wc1": [128, 512],
    "ws0": [128, 512], "ws1": [128, 512],
    "wn0": [128, 1], "wn1": [128, 1],
    "ones4": [128, 32], "ident": [128, 128],
}


def _seg(x_d, b, start, nrows):
    """[nrows, 256] DRAM view of x row b at sample offset `start`."""
    return x_d[b:b + 1, start:start + 256 * nrows].rearrange(
        "o (m r) -> (o m) r", r=256)


def build_nc(repeat=1):
    nc = bacc.Bacc("TRN2", target_bir_lowering=False, debug=False,
                   num_devices=NCORES)

    x_d = {s: nc.dram_tensor(f"x{s}", [RPC, L], dt.float32,
                             kind="ExternalInput") for s in "stg"}
    c_d = {n: nc.dram_tensor(n, shp, dt.float32, kind="ExternalInput")
           for n, shp in CONST_SPECS.items()}
    osum_d = nc.dram_tensor("osum", [RPC * 3, NPF, NPT], dt.float32,
                            kind="ExternalOutput")

    with tile.TileContext(nc) as tc:
        with (
            tc.tile_pool(name="const", bufs=1) as cp,
            tc.tile_pool(name="work", bufs=2) as wp,
            tc.tile_pool(name="upool", bufs=4) as up,
            tc.tile_pool(name="mpool", bufs=13) as mp,
            tc.tile_pool(name="mnpool", bufs=2) as mnp,
            tc.tile_pool(name="dft_ps", bufs=3, space="PSUM") as dft_ps,
            tc.tile_pool(name="tr_ps", bufs=2, space="PSUM") as tr_ps,
            tc.tile_pool(name="ny_ps", bufs=1, space="PSUM") as ny_ps,
            tc.tile_pool(name="pa_ps", bufs=2, space="PSUM") as pa_ps,
        ):
            C = {n: cp.tile(shp, dt.float32, tag=n, name=f"c_{n}")
                 for n, shp in CONST_SPECS.items()}
            for n in CONST_SPECS:
                nc.sync.dma_start(C[n][:], c_d[n][:])

            def tr_to_v(u_ap, v0, v1, col, nr):
                """Transpose [nr, 256] rows into V columns [col, col+nr)."""
                for h, vt in ((0, v0), (1, v1)):
                    tp = tr_ps.tile([128, 128], dt.float32, tag="trp",
                                    name="tp")
                    nc.tensor.transpose(
                        tp[:, 0:nr], u_ap[:, 128 * h:128 * h + 128],
                        C["ident"][0:nr, 0:nr])
                    nc.scalar.copy(vt[:, col:col + nr], tp[:, 0:nr])

            def rev2(name, hi_start, lo_start, s, b):
                """[2,256] tile: row0=rev(x[hi:hi+256]), row1=rev(x[lo:..])."""
                sc = up.tile([2, 256], dt.float32, tag="sc", name="sc")
                nc.sync.dma_start(sc[0:1, :],
                                  x_d[s][b:b + 1, hi_start:hi_start + 256])
                nc.sync.dma_start(sc[1:2, :],
                                  x_d[s][b:b + 1, lo_start:lo_start + 256])
                ur = up.tile([2, 256], dt.float32, tag="ur", name="ur")
                nc.vector.tensor_copy(ur[:], sc[0:2, 255::-1])
                return ur

            def build_V(s, b):
                """Load row b of signal s, reflect-pad, transpose to
                V[r, m] (two [128, NBLK] tiles, r-halves)."""
                v0 = wp.tile([128, NBLK], dt.float32, tag="v0")
                v1 = wp.tile([128, NBLK], dt.float32, tag="v1")
                # head reflect: U[0,r]=x[512-r]=rev(x[257:513]);
                #               U[1,r]=x[256-r]=rev(x[1:257])
                uh = rev2("uh", 257, 1, s, b)
                tr_to_v(uh[0:2, :], v0, v1, 0, 2)
                u0 = up.tile([128, 256], dt.float32, tag="u", name="u0")
                nc.sync.dma_start(u0[0:126, :], _seg(x_d[s], b, 0, 126))
                tr_to_v(u0[0:126, :], v0, v1, 2, 126)
                for i in range(1, 8):
                    u = up.tile([128, 256], dt.float32, tag="u", name="u")
                    nc.sync.dma_start(
                        u[:], _seg(x_d[s], b, 32768 * i - 512, 128))
                    tr_to_v(u[:], v0, v1, 128 * i, 128)
                # tail: blocks 1024,1025 contiguous
                ut = up.tile([2, 256], dt.float32, tag="ur", name="ut")
                nc.sync.dma_start(ut[:], _seg(x_d[s], b, 261632, 2))
                tr_to_v(ut[0:2, :], v0, v1, 1024, 2)
                # tail reflect: U[1026,r]=x[262142-r]=rev(x[261887:262143]);
                #               U[1027,r]=x[261886-r]=rev(x[261631:261887])
                ub = rev2("ub", 261887, 261631, s, b)
                tr_to_v(ub[0:2, :], v0, v1, 1026, 2)
                return v0, v1

            def stft_mag(s, b):
                """mag chunks [128, T] x4 (freq k=4p+c) + nyquist [1, T]."""
                v0, v1 = build_V(s, b)
                mags = []
                for c in range(4):
                    rc = wp.tile([128, NBLK], dt.float32, tag="rc")
                    ic = wp.tile([128, NBLK], dt.float32, tag="ic")
                    for (w0, w1, dst) in (("wc0", "wc1", rc),
                                          ("ws0", "ws1", ic)):
                        for lo, hi in M_RANGES:
                            ps = dft_ps.tile([128, hi - lo], dt.float32,
                                             tag="dftp")
                            nc.tensor.matmul(
                                ps[:], C[w0][:, 128 * c:128 * (c + 1)],
                                v0[:, lo:hi], start=True, stop=False)
                            nc.tensor.matmul(
                                ps[:], C[w1][:, 128 * c:128 * (c + 1)],
                                v1[:, lo:hi], start=False, stop=True)
                            nc.scalar.copy(dst[:, lo:hi], ps[:])
                    # radix-4 recombination (pure adds)
                    pr = wp.tile([128, NBLK - 2], dt.float32, tag="pr")
                    pi = wp.tile([128, NBLK - 2], dt.float32, tag="pi")
                    op2 = nc.vector.tensor_add if c % 2 == 0 else \
                        nc.vector.tensor_sub
                    op2(pr[:], rc[:, 0:NBLK - 2], rc[:, 2:NBLK])
                    op2(pi[:], ic[:, 0:NBLK - 2], ic[:, 2:NBLK])
                    xre = wp.tile([128, T], dt.float32, tag="xre")
                    xim = wp.tile([128, T], dt.float32, tag="xim")
                    if c == 0:
                        nc.vector.tensor_add(xre[:], pr[:, 0:T], pr[:, 1:T + 1])
                        nc.vector.tensor_add(xim[:], pi[:, 0:T], pi[:, 1:T + 1])
                    elif c == 2:
                        nc.vector.tensor_sub(xre[:], pr[:, 0:T], pr[:, 1:T + 1])
                        nc.vector.tensor_sub(xim[:], pi[:, 0:T], pi[:, 1:T + 1])
                    elif c == 1:
                        nc.vector.tensor_add(xre[:], pr[:, 0:T], pi[:, 1:T + 1])
                        nc.vector.tensor_sub(xim[:], pi[:, 0:T], pr[:, 1:T + 1])
                    else:
                        nc.vector.tensor_sub(xre[:], pr[:, 0:T], pi[:, 1:T + 1])
                        nc.vector.tensor_add(xim[:], pi[:, 0:T], pr[:, 1:T + 1])
                    # in-place: xre <- xre^2, xim <- xim^2, xre <- xre+xim
                    nc.scalar.activation(xre[:], xre[:], Act.Square)
                    nc.scalar.activation(xim[:], xim[:], Act.Square)
                    nc.vector.tensor_add(xre[:], xre[:], xim[:])
                    mg = mp.tile([128, T], dt.float32, tag="mag")
                    nc.scalar.activation(mg[:], xre[:], Act.Sqrt)
                    mags.append(mg)
                # Nyquist row: k=512, im = 0, class-0 recombination
                bn = mnp.tile([1, NBLK], dt.float32, tag="bn", bufs=2)
                for lo, hi in M_RANGES:
                    ps = ny_ps.tile([1, hi - lo], dt.float32, tag="nyp")
                    nc.tensor.matmul(ps[:], C["wn0"][:, 0:1], v0[:, lo:hi],
                                     start=True, stop=False)
                    nc.tensor.matmul(ps[:], C["wn1"][:, 0:1], v1[:, lo:hi],
                                     start=False, stop=True)
                    nc.vector.tensor_copy(bn[0:1, lo:hi], ps[:])
                an = mnp.tile([1, NBLK - 2], dt.float32, tag="an", bufs=2)
                nc.vector.tensor_add(an[:], bn[0:1, 0:NBLK - 2],
                                     bn[0:1, 2:NBLK])
                mn = mnp.tile([1, T], dt.float32, tag="magn", bufs=4)
                nc.vector.tensor_add(mn[:], an[0:1, 0:T], an[0:1, 1:T + 1])
                nc.scalar.activation(mn[:], mn[:], Act.Abs)
                return mags, mn

            def patch_map(b, mi, ma, mb, na, nb, square):
                """osum[b*3+mi] = per-patch sums of |A-B| or (A-B)^2."""
                pps = pa_ps.tile([32, NPT], dt.float32, tag="pps")
                for c in range(4):
                    d = wp.tile([128, T], dt.float32, tag="d")
                    nc.vector.tensor_sub(d[:], ma[c][:], mb[c][:])
                    if square:
                        nc.scalar.activation(d[:], d[:], Act.Square)
                        src, ab = d, False
                    else:
                        src, ab = d, True
                    red = wp.tile([128, NPT], dt.float32, tag="red")
                    nc.vector.tensor_reduce(
                        red[:, 0:64],
                        src[:, 0:1024].rearrange("p (a t) -> p a t", t=16),
                        axis=mybir.AxisListType.X, op=Alu.add,
                        apply_absolute_value=ab)
                    nc.vector.tensor_reduce(
                        red[:, 64:65], src[:, 1024:1025],
                        axis=mybir.AxisListType.X, op=Alu.add,
                        apply_absolute_value=ab)
                    nc.tensor.matmul(pps[:], C["ones4"][:], red[:],
                                     start=(c == 0), stop=(c == 3))
                # nyquist row -> patch row 32
                dn = mnp.tile([1, T], dt.float32, tag="dn", bufs=2)
                nc.vector.tensor_sub(dn[:], na[:], nb[:])
                if square:
                    nc.scalar.activation(dn[:], dn[:], Act.Square)
                    srcn, ab = dn, False
                else:
                    srcn, ab = dn, True
                outt = wp.tile([NPF, NPT], dt.float32, tag="outt")
                nc.vector.tensor_reduce(
                    outt[32:33, 0:64],
                    srcn[0:1, 0:1024].rearrange("p (a t) -> p a t", t=16),
                    axis=mybir.AxisListType.X, op=Alu.add,
                    apply_absolute_value=ab)
                nc.vector.tensor_reduce(
                    outt[32:33, 64:65], srcn[0:1, 1024:1025],
                    axis=mybir.AxisListType.X, op=Alu.add,
                    apply_absolute_value=ab)
                nc.vector.tensor_copy(outt[0:32, :], pps[:])
                idx = b * 3 + mi
                nc.sync.dma_start(
                    osum_d[idx:idx + 1].rearrange("o p f -> (o p) f"),
                    outt[:])

            def body():
                for b in range(RPC):
                    ms, nys = stft_mag("s", b)
                    mt, nyt = stft_mag("t", b)
                    mg_, nyg = stft_mag("g", b)
                    patch_map(b, 0, ms, mg_, nys, nyg, False)
                    patch_map(b, 1, mt, mg_, nyt, nyg, False)
                    patch_map(b, 2, ms, mt, nys, nyt, True)

            if repeat == 1:
                body()
            else:
                with tc.For_i(0, repeat, 1):
                    body()

    nc.compile()
    return nc


_NC_CACHE = {}


def _get_nc():
    if "nc" not in _NC_CACHE:
        _NC_CACHE["nc"] = build_nc()
    return _NC_CACHE["nc"]


def _run_on_cores(nc, in_maps):
    """Execute via cached PJRT callable (axon) with jit reuse."""
    from concourse.bass_utils import axon_active

    if not axon_active():
        from concourse.bass_utils import run_bass_kernel_spmd
        return run_bass_kernel_spmd(nc, in_maps,
                                    core_ids=list(range(NCORES))).results

    import jax
    from jax.sharding import Mesh, PartitionSpec
    from jax.experimental.shard_map import shard_map
    from concourse import bass2jax

    key = id(nc)
    if key not in _NC_CACHE.setdefault("jit", {}):
        bass2jax.install_neuronx_cc_hook()
        part_name = (nc.partition_id_tensor.name
                     if nc.partition_id_tensor else None)
        in_names, out_names, out_avals, zero_outs = [], [], [], []
        for alloc in nc.m.functions[0].allocations:
            if not isinstance(alloc, mybir.MemoryLocationSet):
                continue
            name = alloc.memorylocations[0].name
            if alloc.kind == "ExternalInput":
                if name != part_name:
                    in_names.append(name)
            elif alloc.kind == "ExternalOutput":
                shape = tuple(alloc.tensor_shape)
                dtype = mybir.dt.np(alloc.dtype)
                out_names.append(name)
                out_avals.append(jax.core.ShapedArray(shape, dtype))
                zero_outs.append(np.zeros(shape, dtype))
        n_params = len(in_names)
        all_names = in_names + out_names
        if part_name is not None:
            all_names = all_names + [part_name]

        def _body(*args):
            operands = list(args)
            if part_name is not None:
                operands.append(bass2jax.partition_id_tensor())
            outs = bass2jax._bass_exec_p.bind(
                *operands, out_avals=tuple(out_avals),
                in_names=tuple(all_names), out_names=tuple(out_names),
                lowering_input_output_aliases=(),
                sim_require_finite=True, sim_require_nnan=True, nc=nc)
            return tuple(outs)

        devices = jax.devices()[:NCORES]
        mesh = Mesh(np.asarray(devices), ("core",))
        n_outs = len(out_names)
        sharded = jax.jit(
            shard_map(_body, mesh=mesh,
                      in_specs=(PartitionSpec("core"),) * (n_params + n_outs),
                      out_specs=(PartitionSpec("core"),) * n_outs,
                      check_rep=False),
            donate_argnums=tuple(range(n_params, n_params + n_outs)),
            keep_unused=True)
        _NC_CACHE["jit"][key] = (sharded, in_names, out_names, out_avals,
                                 zero_outs)

    sharded, in_names, out_names, out_avals, zero_outs = _NC_CACHE["jit"][key]
    concat_in = [np.concatenate([m[n] for m in in_maps], axis=0)
                 for n in in_names]
    concat_zeros = [np.zeros((NCORES * z.shape[0], *z.shape[1:]), z.dtype)
                    for z in zero_outs]
    out_arrs = sharded(*concat_in, *concat_zeros)
    return [
        {n: np.asarray(out_arrs[i]).reshape(NCORES, *out_avals[i].shape)[c]
         for i, n in enumerate(out_names)}
        for c in range(NCORES)
    ]


def kernel(student_waveform, teacher_waveform, target_waveform,
           n_fft=1024, hop_length=256, patch_size=16):
    xs = np.ascontiguousarray(student_waveform, dtype=np.float32)
    xt = np.ascontiguousarray(teacher_waveform, dtype=np.float32)
    xg = np.ascontiguousarray(target_waveform, dtype=np.float32)

    nc = _get_nc()
    consts = _consts()
    in_maps = []
    for c in range(NCORES):
        m = {"xs": xs[RPC * c:RPC * (c + 1)],
             "xt": xt[RPC * c:RPC * (c + 1)],
             "xg": xg[RPC * c:RPC * (c + 1)]}
        m.update(consts)
        in_maps.append(m)

    results = _run_on_cores(nc, in_maps)

    # [B, 3, NPF, NPT] patch sums
    osum = np.concatenate(
        [r["osum"].reshape(RPC, 3, NPF, NPT) for r in results], axis=0)
    sums = osum.reshape(B, 3, NPF * NPT).astype(np.float32)
    inv = np.float32(1.0 / (PS * PS))
    err_s = sums[:, 0] * inv
    err_t = sums[:, 1] * inv
    pl = sums[:, 2] * inv
    kgs = err_s - err_t

    order = np.argsort(-kgs, axis=1, kind="stable")[:, :KSEL]
    mask = np.zeros_like(kgs)
    np.put_along_axis(mask, order, 1.0, axis=1)
    selected = (pl * mask).sum(axis=1, dtype=np.float32)
    count = np.maximum(mask.sum(axis=1, dtype=np.float32), 1.0)
    loss = np.float32(np.mean(selected / count, dtype=np.float32))
    sel_ratio = np.float32(mask.mean(dtype=np.float32))
    kgs_mean = np.float32(kgs.mean(dtype=np.float32))
    kgs_pos_ratio = np.float32((kgs > 0).mean(dtype=np.float32))
    return loss, sel_ratio, kgs_mean, kgs_pos_ratio
